# revision 4
# baseline (speedup 1.0000x reference)
"""Trainium2 Bass kernel for a OneBlob-encoded 3-layer MLP (ConditioningNetwork).

Math:  x = clip(concat(pos01, wi01, rough01), 0, 1)          [N, 7]
       enc[n, d*32+j] = exp(-0.5 ((x[n,d]-c[j]) / sigma)^2)  [N, 224], sigma = 1/32
       y = relu(relu(enc@W1+b1)@W2+b2)@W3+b3                 [N, 64]

Strategy (pure data parallel over 8 cores, weights replicated):
  - The Gaussian exponent z = -(x-c)^2/(2 sigma^2) is affine in (x, x^2), so it
    is computed on the PE as a small matmul ("expand"): z = L^T @ [x; x^2; 1].
    x and x^2 are fed as exact fp16 hi+lo pairs with hi/lo weight rows; all
    products are exact when accumulated fp32 in PSUM, giving |dz| ~ 1e-3.
  - This environment's PE runs at 1.2 GHz (HAM never ramps to 2.4GHz), so the
    kernel is PE/ACT/DVE co-limited at ~2.2-2.5us per 1024-ray supertile.
    The schedule below keeps all three engines dense:
    * expand operands are split by enc-feature group: dims 0-3 (22 rows ->
      enc[0:128], "hi") on PE row strips 0/1 for the A/B ray halves, and
      dims 4-6 (17 rows -> enc[128:224], "lo") on strips 2/3.
    * the zhi expand pair (strips 0,1 x all cols) runs CONCURRENTLY with
      L2-B/L3-B (strips 2,3); the zlo pair (strips 2,3) runs concurrently
      with L2-A/L3-A (strips 0,1).  L2/L3 cost almost no extra PE time.
    * zhi [128,1024] and zlo [96,1024] rotate through one 2-buffer PSUM pool
      (4 banks): exp-hi(t) frees zhi's slot while exp-lo(t) still runs, so
      expand(t+1) overlaps exp(t) and ACT stays ~100% busy.
    * deep pipeline lags (L1@t+2, h1s@t+3, L2@t+4, L3@t+5) make every
      cross-engine dependency >=1 iteration old - no rendezvous stalls.
    * expand matmuls are emitted at scheduler priority 0 so they dispatch
      the moment their PSUM slot frees.
  - Bias+ReLU are single DVE tensor_scalar ops over packed [128, 512] PSUM
    tiles (A rays on partitions 0:64, B rays on 64:128).
  - Output is feature-major packed [128, 512] per supertile (A rays on
    partitions 64:128 via L3's flipped quadrants), unpacked on the host.

Input row packing (fp16, 128 rows):
  rows  0:22   G03: for d in 0..3: [x_hi, x_hi, x_lo, q_hi, q_lo] (q = x^2),
               then 2 ones rows (u_hi/u_lo constant-term weights)
  rows 32:54   copy of G03  (B-half expand strip)
  rows 64:81   G46: same for d in 4..6, then 2 ones rows
  rows 96:113  copy of G46
"""

import sys

import numpy as np

if "/opt/trn_rl_repo" not in sys.path:
    sys.path.insert(0, "/opt/trn_rl_repo")

N_CORES = 8
N_TOTAL = 1048576
NC_RAYS = N_TOTAL // N_CORES  # 131072 rays per core
BINS = 32
HID = 64
OUT = 64
IN_DIMS = 7
ENC = IN_DIMS * BINS  # 224
SIGMA = 1.0 / BINS

XROWS = 128
B = 512  # rays per matmul (one fp32 PSUM bank)
SUPER = 2 * B  # rays per supertile (A/B halves)
G = 8  # supertiles per DMA group
GROUP_RAYS = SUPER * G  # 8192
N_GROUPS = NC_RAYS // GROUP_RAYS  # 16

# Expand operand row layout
R03 = 22  # rows in the dims 0-3 group (4*5 + 2 ones)
R46 = 17  # rows in the dims 4-6 group (3*5 + 2 ones)
ENC_HI = 128  # enc features from dims 0-3
ENC_LO = ENC - ENC_HI  # 96, from dims 4-6

# Set by the last kernel() call so a test harness can read profile/exec time.
LAST_RESULTS = None

_BUILD_CACHE = {}


def _build_bass(nc_rays, n_groups):
    import concourse.tile as tile
    from concourse import bacc, mybir

    dt = mybir.dt
    Act = mybir.ActivationFunctionType
    Alu = mybir.AluOpType

    nc = bacc.Bacc("TRN2", target_bir_lowering=False, debug=False)

    n_super = n_groups * G

    xp = nc.dram_tensor("xp", [XROWS, nc_rays], dt.float16, kind="ExternalInput")
    lw = nc.dram_tensor("lw", [XROWS, ENC_HI], dt.float16, kind="ExternalInput")
    w1a = nc.dram_tensor("w1a", [ENC_HI, HID], dt.float16, kind="ExternalInput")
    w1b = nc.dram_tensor("w1b", [ENC_LO, HID], dt.float16, kind="ExternalInput")
    w2s = nc.dram_tensor("w2s", [128, HID], dt.float16, kind="ExternalInput")
    w3s = nc.dram_tensor("w3s", [128, OUT], dt.float16, kind="ExternalInput")
    b1s = nc.dram_tensor("b1s", [128, 1], dt.float32, kind="ExternalInput")
    b2s = nc.dram_tensor("b2s", [128, 1], dt.float32, kind="ExternalInput")
    b3s = nc.dram_tensor("b3s", [128, 1], dt.float32, kind="ExternalInput")
    # Output, packed per supertile: rows 64:128 = A-half rays (first 512),
    # rows 0:64 = B-half rays (last 512).
    yt = nc.dram_tensor("yt", [128, nc_rays // 2], dt.float32, kind="ExternalOutput")

    with tile.TileContext(nc) as tc:
        with (
            tc.tile_pool(name="consts", bufs=1) as consts,
            tc.tile_pool(name="xpool", bufs=3) as xpool,
            tc.tile_pool(name="encp", bufs=3) as encp,
            tc.tile_pool(name="hp", bufs=3) as hp,
            tc.tile_pool(name="outp", bufs=3) as outp,
            tc.tile_pool(name="zp", bufs=2, space="PSUM") as zp,
            tc.tile_pool(name="ph", bufs=4, space="PSUM") as ph,
        ):
            lw_t = consts.tile([XROWS, ENC_HI], dt.float16, tag="lw_t")
            nc.sync.dma_start(out=lw_t[:], in_=lw[:])
            w1a_t = consts.tile([ENC_HI, HID], dt.float16, tag="w1a_t")
            nc.sync.dma_start(out=w1a_t[:], in_=w1a[:])
            w1b_t = consts.tile([ENC_LO, HID], dt.float16, tag="w1b_t")
            nc.sync.dma_start(out=w1b_t[:], in_=w1b[:])
            w2s_t = consts.tile([128, HID], dt.float16, tag="w2s_t")
            nc.sync.dma_start(out=w2s_t[:], in_=w2s[:])
            w3s_t = consts.tile([128, OUT], dt.float16, tag="w3s_t")
            nc.sync.dma_start(out=w3s_t[:], in_=w3s[:])
            b1s_t = consts.tile([128, 1], dt.float32, tag="b1s_t")
            nc.sync.dma_start(out=b1s_t[:], in_=b1s[:])
            b2s_t = consts.tile([128, 1], dt.float32, tag="b2s_t")
            nc.sync.dma_start(out=b2s_t[:], in_=b2s[:])
            b3s_t = consts.tile([128, 1], dt.float32, tag="b3s_t")
            nc.sync.dma_start(out=b3s_t[:], in_=b3s[:])

            xts = {}   # group -> xt tile
            encs = {}  # supertile -> [ehi, elo]
            h1ps = {}  # supertile -> h1 PSUM tile
            h1ss = {}  # supertile -> h1s SBUF tile
            h2ps = {}  # supertile -> h2 PSUM tile
            h2ss = {}  # supertile -> h2s SBUF tile
            opts = {}  # supertile -> op PSUM tile

            def ensure_group(g):
                if g in xts or g >= n_groups:
                    return
                g0 = g * GROUP_RAYS
                xt = xpool.tile([XROWS, GROUP_RAYS], dt.float16, tag="xt",
                                name=f"xt{g}")
                nc.sync.dma_start(out=xt[:], in_=xp[:, g0 : g0 + GROUP_RAYS])
                xts[g] = xt

            def ray_cols(t):
                g, j = divmod(t, G)
                ca = slice(j * SUPER, j * SUPER + B)
                cb = slice(j * SUPER + B, (j + 1) * SUPER)
                return xts[g], ca, cb

            def emit_expand_hi(t):
                """A+B hi expands on row strips 0 and 1 (concurrent)."""
                xt, ca, cb = ray_cols(t)
                zhi = zp.tile([128, SUPER], dt.float32, tag="z", name=f"zhi{t}")
                with tc.high_priority():
                    nc.tensor.matmul(
                        zhi[:, 0:B], lhsT=lw_t[0:R03, :], rhs=xt[0:R03, ca],
                        start=True, stop=True, tile_position=(0, 0),
                    )
                    nc.tensor.matmul(
                        zhi[:, B : 2 * B], lhsT=lw_t[32 : 32 + R03, :],
                        rhs=xt[32 : 32 + R03, cb],
                        start=True, stop=True, tile_position=(32, 0),
                    )
                ehi = encp.tile([128, SUPER], dt.float16, tag="ehi",
                                name=f"ehi{t}")
                nc.scalar.activation(ehi[:], zhi[:], Act.Exp)
                encs.setdefault(t, [None, None])[0] = ehi

            def emit_expand_lo(t):
                """A+B lo expands on row strips 2 and 3 (concurrent)."""
                xt, ca, cb = ray_cols(t)
                zlo = zp.tile([128, SUPER], dt.float32, tag="z", name=f"zlo{t}")
                with tc.high_priority():
                    nc.tensor.matmul(
                        zlo[0:ENC_LO, 0:B], lhsT=lw_t[64 : 64 + R46, 0:ENC_LO],
                        rhs=xt[64 : 64 + R46, ca],
                        start=True, stop=True, tile_position=(64, 0),
                    )
                    nc.tensor.matmul(
                        zlo[0:ENC_LO, B : 2 * B],
                        lhsT=lw_t[96 : 96 + R46, 0:ENC_LO],
                        rhs=xt[96 : 96 + R46, cb],
                        start=True, stop=True, tile_position=(96, 0),
                    )
                elo = encp.tile([ENC_LO, SUPER], dt.float16, tag="elo",
                                name=f"elo{t}")
                nc.scalar.activation(elo[:], zlo[0:ENC_LO, :], Act.Exp)
                encs[t][1] = elo

            def emit_l1(t):
                ehi, elo = encs.pop(t)
                h1 = ph.tile([128, B], dt.float32, tag="hh", name=f"h1_{t}")
                nc.tensor.matmul(h1[0:64, :], lhsT=w1a_t[:], rhs=ehi[:, 0:B],
                                 start=True, stop=False, tile_position=(0, 0))
                nc.tensor.matmul(h1[64:128, :], lhsT=w1a_t[:],
                                 rhs=ehi[:, B : 2 * B],
                                 start=True, stop=False, tile_position=(0, 64))
                nc.tensor.matmul(h1[0:64, :], lhsT=w1b_t[:], rhs=elo[:, 0:B],
                                 start=False, stop=True, tile_position=(0, 0))
                nc.tensor.matmul(h1[64:128, :], lhsT=w1b_t[:],
                                 rhs=elo[:, B : 2 * B],
                                 start=False, stop=True, tile_position=(0, 64))
                h1ps[t] = h1

            def emit_h1s(t):
                h1 = h1ps.pop(t)
                h1s = hp.tile([128, B], dt.float16, tag="h1s", name=f"h1s{t}")
                nc.vector.tensor_scalar(h1s[:], h1[:], b1s_t[:], 0.0,
                                        Alu.add, Alu.max)
                h1ss[t] = h1s

            def emit_l2b(t):
                """B-half L2 on quadrant (64,64) - concurrent with zhi pair."""
                h2 = ph.tile([128, B], dt.float32, tag="hh", name=f"h2_{t}")
                nc.tensor.matmul(h2[64:128, :], lhsT=w2s_t[64:128, :],
                                 rhs=h1ss[t][64:128, :],
                                 start=True, stop=True, tile_position=(64, 64))
                h2ps[t] = h2

            def emit_l2a(t):
                """A-half L2 on quadrant (0,0) - concurrent with zlo pair."""
                nc.tensor.matmul(h2ps[t][0:64, :], lhsT=w2s_t[0:64, :],
                                 rhs=h1ss.pop(t)[0:64, :],
                                 start=True, stop=True, tile_position=(0, 0))

            def emit_h2s(t):
                h2 = h2ps.pop(t)
                h2s = hp.tile([128, B], dt.float16, tag="h2s", name=f"h2s{t}")
                nc.vector.tensor_scalar(h2s[:], h2[:], b2s_t[:], 0.0,
                                        Alu.add, Alu.max)
                h2ss[t] = h2s

            def emit_l3b(t):
                """B-half L3 on quadrant (64,0) - concurrent with zhi pair."""
                op = ph.tile([128, B], dt.float32, tag="hh", name=f"op{t}")
                nc.tensor.matmul(op[0:64, :], lhsT=w3s_t[64:128, :],
                                 rhs=h2ss[t][64:128, :],
                                 start=True, stop=True, tile_position=(64, 0))
                opts[t] = op

            def emit_l3a(t):
                """A-half L3 on quadrant (0,64) - concurrent with zlo pair."""
                nc.tensor.matmul(opts[t][64:128, :], lhsT=w3s_t[0:64, :],
                                 rhs=h2ss.pop(t)[0:64, :],
                                 start=True, stop=True, tile_position=(0, 64))

            def emit_out(t):
                op = opts.pop(t)
                os_ = outp.tile([128, B], dt.float32, tag="os", name=f"os{t}")
                nc.vector.tensor_scalar_add(os_[:], op[:], b3s_t[:])
                nc.sync.dma_start(out=yt[:, t * B : (t + 1) * B], in_=os_[:])

            # Deep pipeline, all cross-engine deps >= 1 iteration old:
            #   PE:  [zhi(t) || L2B(t-4) L3B(t-5)] [zlo(t) || L2A(t-4)
            #        L3A(t-5)] [L1(t-2)]
            #   ACT: exp-hi(t), exp-lo(t)
            #   DVE: h1s(t-3), h2s(t-4), os(t-5)
            for t in range(n_super + 5):
                if t < n_super:
                    ensure_group(t // G)
                    emit_expand_hi(t)
                if 0 <= t - 3 < n_super:
                    emit_h1s(t - 3)
                if 0 <= t - 4 < n_super:
                    emit_l2b(t - 4)
                if 0 <= t - 5 < n_super:
                    emit_l3b(t - 5)
                if t < n_super:
                    emit_expand_lo(t)
                if 0 <= t - 4 < n_super:
                    emit_l2a(t - 4)
                if 0 <= t - 5 < n_super:
                    emit_l3a(t - 5)
                if 0 <= t - 4 < n_super:
                    emit_h2s(t - 4)
                if 0 <= t - 5 < n_super:
                    emit_out(t - 5)
                if 0 <= t - 2 < n_super:
                    emit_l1(t - 2)

    nc.finalize()
    return nc


def _get_nc():
    key = (NC_RAYS, N_GROUPS)
    if key not in _BUILD_CACHE:
        _BUILD_CACHE[key] = _build_bass(*key)
    return _BUILD_CACHE[key]


def _f16_hilo(x64):
    """Exact hi/lo split: x ~= hi + lo with hi, lo fp16 (inputs are fp64)."""
    hi = x64.astype(np.float16)
    lo = (x64 - hi.astype(np.float64)).astype(np.float16)
    return hi, lo


def _expand_weight_rows():
    """Per-dim weight rows for the expand matmul (on the 32 enc bins).

    z = -inv2s2*x^2 + (2*inv2s2*c_j)*x - inv2s2*c_j^2, rows pair with
    [x_hi, x_hi, x_lo, q_hi, q_lo] and two trailing ones rows.
    """
    c = np.linspace(0.0, 1.0, BINS).astype(np.float64)
    inv2s2 = 0.5 / (SIGMA * SIGMA)  # 512
    wx = 2.0 * inv2s2 * c
    wq = -inv2s2
    wu = -inv2s2 * c * c
    wx_hi = wx.astype(np.float16)
    wx_lo = (wx - wx_hi.astype(np.float64)).astype(np.float16)
    wu_hi = wu.astype(np.float16)
    wu_lo = (wu - wu_hi.astype(np.float64)).astype(np.float16)
    return wx_hi, wx_lo, np.float16(wq), wu_hi, wu_lo


def _pack_weights(W1, b1, W2, b2, W3, b3):
    wx_hi, wx_lo, wq, wu_hi, wu_lo = _expand_weight_rows()

    lw = np.zeros((XROWS, ENC_HI), np.float16)
    # G03: dims 0-3 -> enc cols 0:128
    for d in range(4):
        cols = slice(d * BINS, (d + 1) * BINS)
        lw[5 * d + 0, cols] = wx_hi
        lw[5 * d + 1, cols] = wx_lo
        lw[5 * d + 2, cols] = wx_hi
        lw[5 * d + 3, cols] = wq
        lw[5 * d + 4, cols] = wq
    lw[20, 0:128] = np.tile(wu_hi, 4)
    lw[21, 0:128] = np.tile(wu_lo, 4)
    # G46: dims 4-6 -> enc cols 128:224 (stored at cols 0:96)
    for d in range(3):
        cols = slice(d * BINS, (d + 1) * BINS)
        lw[64 + 5 * d + 0, cols] = wx_hi
        lw[64 + 5 * d + 1, cols] = wx_lo
        lw[64 + 5 * d + 2, cols] = wx_hi
        lw[64 + 5 * d + 3, cols] = wq
        lw[64 + 5 * d + 4, cols] = wq
    lw[64 + 15, 0:96] = np.tile(wu_hi, 3)
    lw[64 + 16, 0:96] = np.tile(wu_lo, 3)
    # duplicates for the B-half strips
    lw[32:54] = lw[0:22]
    lw[96:113] = lw[64:81]

    w1 = W1.astype(np.float16)
    return {
        "lw": lw,
        "w1a": np.ascontiguousarray(w1[0:ENC_HI]),
        "w1b": np.ascontiguousarray(w1[ENC_HI:ENC]),
        "w2s": np.concatenate([W2, W2], 0).astype(np.float16),
        "w3s": np.concatenate([W3, W3], 0).astype(np.float16),
        "b1s": np.concatenate([b1, b1], 0).astype(np.float32).reshape(128, 1),
        "b2s": np.concatenate([b2, b2], 0).astype(np.float32).reshape(128, 1),
        "b3s": np.concatenate([b3, b3], 0).astype(np.float32).reshape(128, 1),
    }


def _pack_inputs(pos01, wi01, rough01):
    x = np.concatenate(
        [np.asarray(pos01), np.asarray(wi01), np.asarray(rough01)], axis=1
    ).astype(np.float32)
    np.clip(x, 0.0, 1.0, out=x)
    x64 = x.astype(np.float64)
    q64 = x64 * x64
    x_hi, x_lo = _f16_hilo(x64)
    q_hi, q_lo = _f16_hilo(q64)
    n = x.shape[0]
    xp = np.zeros((XROWS, n), np.float16)

    def fill(base, dims):
        for i, d in enumerate(dims):
            xp[base + 5 * i + 0] = x_hi[:, d]
            xp[base + 5 * i + 1] = x_hi[:, d]
            xp[base + 5 * i + 2] = x_lo[:, d]
            xp[base + 5 * i + 3] = q_hi[:, d]
            xp[base + 5 * i + 4] = q_lo[:, d]
        xp[base + 5 * len(dims)] = np.float16(1.0)
        xp[base + 5 * len(dims) + 1] = np.float16(1.0)

    fill(0, (0, 1, 2, 3))
    fill(64, (4, 5, 6))
    xp[32:54] = xp[0:22]
    xp[96:113] = xp[64:81]
    return xp


def kernel(pos01, wi01, rough01, W1, b1, W2, b2, W3, b3, centers):
    global LAST_RESULTS
    import os

    from concourse.bass_utils import run_bass_kernel_spmd

    nc = _get_nc()

    xp = _pack_inputs(pos01, wi01, rough01)
    wpacks = _pack_weights(
        np.asarray(W1), np.asarray(b1), np.asarray(W2), np.asarray(b2),
        np.asarray(W3), np.asarray(b3),
    )

    in_maps = []
    for c in range(N_CORES):
        m = dict(wpacks)
        m["xp"] = np.ascontiguousarray(xp[:, c * NC_RAYS : (c + 1) * NC_RAYS])
        in_maps.append(m)

    trace = bool(int(os.environ.get("KERNEL_TRACE", "0")))
    res = run_bass_kernel_spmd(nc, in_maps, list(range(N_CORES)), trace=trace)
    LAST_RESULTS = res

    out = np.empty((N_TOTAL, OUT), np.float32)
    for c in range(N_CORES):
        yt = res.results[c]["yt"]  # [128, NC_RAYS // 2]
        # Per supertile t: cols [t*512:(t+1)*512]; rows 64:128 = A rays
        # (rays t*1024 .. +512), rows 0:64 = B rays (last 512).
        n_super = NC_RAYS // SUPER
        a = yt[OUT:128].reshape(OUT, n_super, B)
        b = yt[0:OUT].reshape(OUT, n_super, B)
        stacked = np.stack([a, b], axis=2)  # [64, t, 2, 512]
        out[c * NC_RAYS : (c + 1) * NC_RAYS] = (
            stacked.transpose(1, 2, 3, 0).reshape(NC_RAYS, OUT)
        )
    return out


# revision 15
# speedup vs baseline: 1.2733x; 1.2733x over previous
"""Trainium2 Bass kernel for a OneBlob-encoded 3-layer MLP (ConditioningNetwork).

Math:  x = clip(concat(pos01, wi01, rough01), 0, 1)          [N, 7]
       enc[n, d*32+j] = exp(-0.5 ((x[n,d]-c[j]) / sigma)^2)  [N, 224], sigma = 1/32
       y = relu(relu(enc@W1+b1)@W2+b2)@W3+b3                 [N, 64]

Strategy (pure data parallel over 8 cores, weights replicated):
  - The Gaussian exponent z = -(x-c)^2/(2 sigma^2) is affine in (x, x^2), so it
    is computed on the PE as a small matmul ("expand"): z = L^T @ [x; x^2; 1].
    x and x^2 are fed as exact fp16 hi+lo pairs with hi/lo weight rows; all
    products are exact when accumulated fp32 in PSUM, giving |dz| ~ 1e-3.
  - This environment's PE runs at 1.2 GHz (HAM never ramps to 2.4GHz), so the
    kernel is PE/ACT/DVE co-limited at ~2.2-2.5us per 1024-ray supertile.
    The schedule below keeps all three engines dense:
    * expand operands are split by enc-feature group: dims 0-3 (22 rows ->
      enc[0:128], "hi") on PE row strips 0/1 for the A/B ray halves, and
      dims 4-6 (17 rows -> enc[128:224], "lo") on strips 2/3.
    * the zhi expand pair (strips 0,1 x all cols) runs CONCURRENTLY with
      L2-B/L3-B (strips 2,3); the zlo pair (strips 2,3) runs concurrently
      with L2-A/L3-A (strips 0,1).  L2/L3 cost almost no extra PE time.
    * zhi [128,1024] and zlo [96,1024] rotate through one 2-buffer PSUM pool
      (4 banks): exp-hi(t) frees zhi's slot while exp-lo(t) still runs, so
      expand(t+1) overlaps exp(t) and ACT stays ~100% busy.
    * deep pipeline lags (L1@t+2, h1s@t+3, L2@t+4, L3@t+5) make every
      cross-engine dependency >=1 iteration old - no rendezvous stalls.
    * expand matmuls are emitted at scheduler priority 0 so they dispatch
      the moment their PSUM slot frees.
  - Bias+ReLU are single DVE tensor_scalar ops over packed [128, 512] PSUM
    tiles (A rays on partitions 0:64, B rays on 64:128).
  - Output is feature-major packed [128, 512] per supertile (A rays on
    partitions 64:128 via L3's flipped quadrants), unpacked on the host.

Input row packing (fp16, 128 rows):
  rows  0:22   G03: for d in 0..3: [x_hi, x_hi, x_lo, q_hi, q_lo] (q = x^2),
               then 2 ones rows (u_hi/u_lo constant-term weights)
  rows 32:54   copy of G03  (B-half expand strip)
  rows 64:81   G46: same for d in 4..6, then 2 ones rows
  rows 96:113  copy of G46
"""

import sys

import numpy as np

if "/opt/trn_rl_repo" not in sys.path:
    sys.path.insert(0, "/opt/trn_rl_repo")

N_CORES = 8
N_TOTAL = 1048576
NC_RAYS = N_TOTAL // N_CORES  # 131072 rays per core
BINS = 32
HID = 64
OUT = 64
IN_DIMS = 7
ENC = IN_DIMS * BINS  # 224
SIGMA = 1.0 / BINS

XROWS = 128
B = 512  # rays per matmul (one fp32 PSUM bank)
SUPER = 2 * B  # rays per supertile (A/B halves)
G = 8  # supertiles per DMA group
GROUP_RAYS = SUPER * G  # 8192
N_GROUPS = NC_RAYS // GROUP_RAYS  # 16

# Expand operand row layout
R03 = 22  # rows in the dims 0-3 group (4*5 + 2 ones)
R46 = 17  # rows in the dims 4-6 group (3*5 + 2 ones)
ENC_HI = 128  # enc features from dims 0-3
ENC_LO = ENC - ENC_HI  # 96, from dims 4-6

# Set by the last kernel() call so a test harness can read profile/exec time.
LAST_RESULTS = None

_BUILD_CACHE = {}


def _build_bass(nc_rays, n_groups):
    import concourse.tile as tile
    from concourse import bacc, mybir

    dt = mybir.dt
    Act = mybir.ActivationFunctionType
    Alu = mybir.AluOpType

    nc = bacc.Bacc("TRN2", target_bir_lowering=False, debug=False)

    n_super = n_groups * G

    xp = nc.dram_tensor("xp", [XROWS, nc_rays], dt.float16, kind="ExternalInput")
    lw = nc.dram_tensor("lw", [XROWS, ENC_HI], dt.float16, kind="ExternalInput")
    w1a = nc.dram_tensor("w1a", [ENC_HI, HID], dt.float16, kind="ExternalInput")
    w1b = nc.dram_tensor("w1b", [ENC_LO, HID], dt.float16, kind="ExternalInput")
    w2s = nc.dram_tensor("w2s", [128, HID], dt.float16, kind="ExternalInput")
    w3s = nc.dram_tensor("w3s", [128, OUT], dt.float16, kind="ExternalInput")
    b1s = nc.dram_tensor("b1s", [128, 1], dt.float32, kind="ExternalInput")
    b2s = nc.dram_tensor("b2s", [128, 1], dt.float32, kind="ExternalInput")
    b3s = nc.dram_tensor("b3s", [128, 1], dt.float32, kind="ExternalInput")
    # Output, packed per supertile: rows 64:128 = A-half rays (first 512),
    # rows 0:64 = B-half rays (last 512).
    yt = nc.dram_tensor("yt", [128, nc_rays // 2], dt.float32, kind="ExternalOutput")

    with tile.TileContext(nc) as tc:
        with (
            tc.tile_pool(name="consts", bufs=1) as consts,
            tc.tile_pool(name="xpool", bufs=3) as xpool,
            tc.tile_pool(name="encp", bufs=4) as encp,
            tc.tile_pool(name="hp", bufs=3) as hp,
            tc.tile_pool(name="outp", bufs=3) as outp,
            tc.tile_pool(name="zp", bufs=2, space="PSUM") as zp,
            tc.tile_pool(name="ph", bufs=4, space="PSUM") as ph,
        ):
            lw_t = consts.tile([XROWS, ENC_HI], dt.float16, tag="lw_t")
            nc.sync.dma_start(out=lw_t[:], in_=lw[:])
            w1a_t = consts.tile([ENC_HI, HID], dt.float16, tag="w1a_t")
            nc.sync.dma_start(out=w1a_t[:], in_=w1a[:])
            w1b_t = consts.tile([ENC_LO, HID], dt.float16, tag="w1b_t")
            nc.sync.dma_start(out=w1b_t[:], in_=w1b[:])
            w2s_t = consts.tile([128, HID], dt.float16, tag="w2s_t")
            nc.sync.dma_start(out=w2s_t[:], in_=w2s[:])
            w3s_t = consts.tile([128, OUT], dt.float16, tag="w3s_t")
            nc.sync.dma_start(out=w3s_t[:], in_=w3s[:])
            b1s_t = consts.tile([128, 1], dt.float32, tag="b1s_t")
            nc.sync.dma_start(out=b1s_t[:], in_=b1s[:])
            b2s_t = consts.tile([128, 1], dt.float32, tag="b2s_t")
            nc.sync.dma_start(out=b2s_t[:], in_=b2s[:])
            b3s_t = consts.tile([128, 1], dt.float32, tag="b3s_t")
            nc.sync.dma_start(out=b3s_t[:], in_=b3s[:])

            xts = {}   # group -> xt tile
            encs = {}  # supertile -> [ehi, elo]
            h1ps = {}  # supertile -> h1 PSUM tile
            h1ss = {}  # supertile -> h1s SBUF tile
            h2ps = {}  # supertile -> h2 PSUM tile
            h2ss = {}  # supertile -> h2s SBUF tile
            opts = {}  # supertile -> op PSUM tile

            def ensure_group(g):
                if g in xts or g >= n_groups:
                    return
                g0 = g * GROUP_RAYS
                xt = xpool.tile([XROWS, GROUP_RAYS], dt.float16, tag="xt",
                                name=f"xt{g}")
                if g == 0:
                    # Per-supertile chunks so compute starts after ~256KB
                    # instead of waiting for the full 2MB group load.
                    for j in range(G):
                        c0 = j * SUPER
                        with tc.tile_wait_until(max(0.0, j - 0.5)):
                            nc.sync.dma_start(
                                out=xt[:, c0 : c0 + SUPER],
                                in_=xp[:, g0 + c0 : g0 + c0 + SUPER],
                            )
                else:
                    with tc.tile_wait_until(max(0.0, g * G - 6.0)):
                        nc.sync.dma_start(
                            out=xt[:], in_=xp[:, g0 : g0 + GROUP_RAYS]
                        )
                xts[g] = xt

            def ray_cols(t):
                g, j = divmod(t, G)
                ca = slice(j * SUPER, j * SUPER + B)
                cb = slice(j * SUPER + B, (j + 1) * SUPER)
                return xts[g], ca, cb

            def emit_expand_hi(t, fl_mm, fl_act):
                """A+B hi expands on row strips 0 and 1 (concurrent)."""
                xt, ca, cb = ray_cols(t)
                zhi = zp.tile([128, SUPER], dt.float32, tag="z", name=f"zhi{t}")
                with tc.tile_wait_until(fl_mm):
                    nc.tensor.matmul(
                        zhi[:, 0:B], lhsT=lw_t[0:R03, :], rhs=xt[0:R03, ca],
                        start=True, stop=True, tile_position=(0, 0),
                    )
                    nc.tensor.matmul(
                        zhi[:, B : 2 * B], lhsT=lw_t[32 : 32 + R03, :],
                        rhs=xt[32 : 32 + R03, cb],
                        start=True, stop=True, tile_position=(32, 0),
                    )
                ehi = encp.tile([128, SUPER], dt.float16, tag="ehi",
                                name=f"ehi{t}")
                with tc.tile_wait_until(fl_act):
                    nc.scalar.activation(ehi[:], zhi[:], Act.Exp)
                encs.setdefault(t, [None, None])[0] = ehi

            def emit_expand_lo(t, fl_mm, fl_act):
                """A+B lo expands on row strips 2 and 3 (concurrent)."""
                xt, ca, cb = ray_cols(t)
                zlo = zp.tile([128, SUPER], dt.float32, tag="z", name=f"zlo{t}")
                with tc.tile_wait_until(fl_mm):
                    nc.tensor.matmul(
                        zlo[0:ENC_LO, 0:B], lhsT=lw_t[64 : 64 + R46, 0:ENC_LO],
                        rhs=xt[64 : 64 + R46, ca],
                        start=True, stop=True, tile_position=(64, 0),
                    )
                    nc.tensor.matmul(
                        zlo[0:ENC_LO, B : 2 * B],
                        lhsT=lw_t[96 : 96 + R46, 0:ENC_LO],
                        rhs=xt[96 : 96 + R46, cb],
                        start=True, stop=True, tile_position=(96, 0),
                    )
                elo = encp.tile([ENC_LO, SUPER], dt.float16, tag="elo",
                                name=f"elo{t}")
                with tc.tile_wait_until(fl_act):
                    nc.scalar.activation(elo[:], zlo[0:ENC_LO, :], Act.Exp)
                encs[t][1] = elo

            def emit_l1(t, fl):
                ehi, elo = encs.pop(t)
                h1 = ph.tile([128, B], dt.float32, tag="hh", name=f"h1_{t}")
                with tc.tile_wait_until(fl):
                    nc.tensor.matmul(h1[0:64, :], lhsT=w1a_t[:],
                                     rhs=ehi[:, 0:B],
                                     start=True, stop=False,
                                     tile_position=(0, 0))
                    nc.tensor.matmul(h1[64:128, :], lhsT=w1a_t[:],
                                     rhs=ehi[:, B : 2 * B],
                                     start=True, stop=False,
                                     tile_position=(0, 64))
                    nc.tensor.matmul(h1[0:64, :], lhsT=w1b_t[:],
                                     rhs=elo[:, 0:B],
                                     start=False, stop=True,
                                     tile_position=(0, 0))
                    nc.tensor.matmul(h1[64:128, :], lhsT=w1b_t[:],
                                     rhs=elo[:, B : 2 * B],
                                     start=False, stop=True,
                                     tile_position=(0, 64))
                h1ps[t] = h1

            def emit_h1s(t, fl):
                h1 = h1ps.pop(t)
                h1s = hp.tile([128, B], dt.float16, tag="h1s", name=f"h1s{t}")
                with tc.tile_wait_until(fl):
                    nc.vector.tensor_scalar(h1s[:], h1[:], b1s_t[:], 0.0,
                                            Alu.add, Alu.max)
                h1ss[t] = h1s

            def emit_l2(t, fl):
                """L2 quadrants (0,0)/(64,64) - batch with L3's flipped pair."""
                h2 = ph.tile([128, B], dt.float32, tag="hh", name=f"h2_{t}")
                h1s = h1ss.pop(t)
                with tc.tile_wait_until(fl):
                    nc.tensor.matmul(h2[0:64, :], lhsT=w2s_t[0:64, :],
                                     rhs=h1s[0:64, :],
                                     start=True, stop=True,
                                     tile_position=(0, 0))
                    nc.tensor.matmul(h2[64:128, :], lhsT=w2s_t[64:128, :],
                                     rhs=h1s[64:128, :],
                                     start=True, stop=True,
                                     tile_position=(64, 64))
                h2ps[t] = h2

            def emit_h2s(t, fl):
                h2 = h2ps.pop(t)
                h2s = hp.tile([128, B], dt.float16, tag="h2s", name=f"h2s{t}")
                with tc.tile_wait_until(fl):
                    nc.vector.tensor_scalar(h2s[:], h2[:], b2s_t[:], 0.0,
                                            Alu.add, Alu.max)
                h2ss[t] = h2s

            def emit_l3(t, fl):
                """L3 on flipped quadrants (0,64)/(64,0); output rows are
                therefore [B-rays; A-rays]."""
                op = ph.tile([128, B], dt.float32, tag="hh", name=f"op{t}")
                h2s = h2ss.pop(t)
                with tc.tile_wait_until(fl):
                    nc.tensor.matmul(op[64:128, :], lhsT=w3s_t[0:64, :],
                                     rhs=h2s[0:64, :],
                                     start=True, stop=True,
                                     tile_position=(0, 64))
                    nc.tensor.matmul(op[0:64, :], lhsT=w3s_t[64:128, :],
                                     rhs=h2s[64:128, :],
                                     start=True, stop=True,
                                     tile_position=(64, 0))
                opts[t] = op

            def emit_out(t, fl):
                op = opts.pop(t)
                os_ = outp.tile([128, B], dt.float32, tag="os", name=f"os{t}")
                with tc.tile_wait_until(fl):
                    nc.vector.tensor_scalar_add(os_[:], op[:], b3s_t[:])
                    nc.sync.dma_start(out=yt[:, t * B : (t + 1) * B],
                                      in_=os_[:])

            # Deep pipeline, all cross-engine deps >= 1 iteration old, with
            # manual-wait floors forcing the per-engine order.  Per block t:
            #   PE:  [L2(t-4) L3(t-5) 4-way] [w1a(t-2)] [w1b(t-2)]
            #        [zlo(t) zhi(t+1) 4-way]
            #   ACT: exp-lo(t), exp-hi(t+1)   (each right after its z pair)
            #   DVE: h2s(t-4), os(t-5), h1s(t-2)
            ensure_group(0)
            emit_expand_hi(0, 0.0, 0.0)
            for t in range(n_super + 5):
                if 0 <= t - 4 < n_super:
                    emit_l2(t - 4, t + 0.00)
                if 0 <= t - 5 < n_super:
                    emit_l3(t - 5, t + 0.01)
                if 0 <= t - 4 < n_super:
                    emit_h2s(t - 4, t + 0.005)
                if 0 <= t - 5 < n_super:
                    emit_out(t - 5, t + 0.015)
                if 0 <= t - 2 < n_super:
                    emit_l1(t - 2, t + 0.02)
                    emit_h1s(t - 2, t + 0.035)
                if t < n_super:
                    emit_expand_lo(t, t + 0.04, t + 0.06)
                if t + 1 < n_super:
                    ensure_group((t + 1) // G)
                    emit_expand_hi(t + 1, t + 0.05, t + 0.07)

    nc.finalize()
    return nc


def _get_nc():
    key = (NC_RAYS, N_GROUPS)
    if key not in _BUILD_CACHE:
        _BUILD_CACHE[key] = _build_bass(*key)
    return _BUILD_CACHE[key]


def _f16_hilo(x64):
    """Exact hi/lo split: x ~= hi + lo with hi, lo fp16 (inputs are fp64)."""
    hi = x64.astype(np.float16)
    lo = (x64 - hi.astype(np.float64)).astype(np.float16)
    return hi, lo


def _expand_weight_rows():
    """Per-dim weight rows for the expand matmul (on the 32 enc bins).

    z = -inv2s2*x^2 + (2*inv2s2*c_j)*x - inv2s2*c_j^2, rows pair with
    [x_hi, x_hi, x_lo, q_hi, q_lo] and two trailing ones rows.
    """
    c = np.linspace(0.0, 1.0, BINS).astype(np.float64)
    inv2s2 = 0.5 / (SIGMA * SIGMA)  # 512
    wx = 2.0 * inv2s2 * c
    wq = -inv2s2
    wu = -inv2s2 * c * c
    wx_hi = wx.astype(np.float16)
    wx_lo = (wx - wx_hi.astype(np.float64)).astype(np.float16)
    wu_hi = wu.astype(np.float16)
    wu_lo = (wu - wu_hi.astype(np.float64)).astype(np.float16)
    return wx_hi, wx_lo, np.float16(wq), wu_hi, wu_lo


def _pack_weights(W1, b1, W2, b2, W3, b3):
    wx_hi, wx_lo, wq, wu_hi, wu_lo = _expand_weight_rows()

    lw = np.zeros((XROWS, ENC_HI), np.float16)
    # G03: dims 0-3 -> enc cols 0:128
    for d in range(4):
        cols = slice(d * BINS, (d + 1) * BINS)
        lw[5 * d + 0, cols] = wx_hi
        lw[5 * d + 1, cols] = wx_lo
        lw[5 * d + 2, cols] = wx_hi
        lw[5 * d + 3, cols] = wq
        lw[5 * d + 4, cols] = wq
    lw[20, 0:128] = np.tile(wu_hi, 4)
    lw[21, 0:128] = np.tile(wu_lo, 4)
    # G46: dims 4-6 -> enc cols 128:224 (stored at cols 0:96)
    for d in range(3):
        cols = slice(d * BINS, (d + 1) * BINS)
        lw[64 + 5 * d + 0, cols] = wx_hi
        lw[64 + 5 * d + 1, cols] = wx_lo
        lw[64 + 5 * d + 2, cols] = wx_hi
        lw[64 + 5 * d + 3, cols] = wq
        lw[64 + 5 * d + 4, cols] = wq
    lw[64 + 15, 0:96] = np.tile(wu_hi, 3)
    lw[64 + 16, 0:96] = np.tile(wu_lo, 3)
    # duplicates for the B-half strips
    lw[32:54] = lw[0:22]
    lw[96:113] = lw[64:81]

    w1 = W1.astype(np.float16)
    return {
        "lw": lw,
        "w1a": np.ascontiguousarray(w1[0:ENC_HI]),
        "w1b": np.ascontiguousarray(w1[ENC_HI:ENC]),
        "w2s": np.concatenate([W2, W2], 0).astype(np.float16),
        "w3s": np.concatenate([W3, W3], 0).astype(np.float16),
        "b1s": np.concatenate([b1, b1], 0).astype(np.float32).reshape(128, 1),
        "b2s": np.concatenate([b2, b2], 0).astype(np.float32).reshape(128, 1),
        "b3s": np.concatenate([b3, b3], 0).astype(np.float32).reshape(128, 1),
    }


def _pack_inputs(pos01, wi01, rough01):
    x = np.concatenate(
        [np.asarray(pos01), np.asarray(wi01), np.asarray(rough01)], axis=1
    ).astype(np.float32)
    np.clip(x, 0.0, 1.0, out=x)
    x64 = x.astype(np.float64)
    q64 = x64 * x64
    x_hi, x_lo = _f16_hilo(x64)
    q_hi, q_lo = _f16_hilo(q64)
    n = x.shape[0]
    xp = np.zeros((XROWS, n), np.float16)

    def fill(base, dims):
        for i, d in enumerate(dims):
            xp[base + 5 * i + 0] = x_hi[:, d]
            xp[base + 5 * i + 1] = x_hi[:, d]
            xp[base + 5 * i + 2] = x_lo[:, d]
            xp[base + 5 * i + 3] = q_hi[:, d]
            xp[base + 5 * i + 4] = q_lo[:, d]
        xp[base + 5 * len(dims)] = np.float16(1.0)
        xp[base + 5 * len(dims) + 1] = np.float16(1.0)

    fill(0, (0, 1, 2, 3))
    fill(64, (4, 5, 6))
    xp[32:54] = xp[0:22]
    xp[96:113] = xp[64:81]
    return xp


def kernel(pos01, wi01, rough01, W1, b1, W2, b2, W3, b3, centers):
    global LAST_RESULTS
    import os

    from concourse.bass_utils import run_bass_kernel_spmd

    nc = _get_nc()

    xp = _pack_inputs(pos01, wi01, rough01)
    wpacks = _pack_weights(
        np.asarray(W1), np.asarray(b1), np.asarray(W2), np.asarray(b2),
        np.asarray(W3), np.asarray(b3),
    )

    in_maps = []
    for c in range(N_CORES):
        m = dict(wpacks)
        m["xp"] = np.ascontiguousarray(xp[:, c * NC_RAYS : (c + 1) * NC_RAYS])
        in_maps.append(m)

    trace = bool(int(os.environ.get("KERNEL_TRACE", "0")))
    res = run_bass_kernel_spmd(nc, in_maps, list(range(N_CORES)), trace=trace)
    LAST_RESULTS = res

    out = np.empty((N_TOTAL, OUT), np.float32)
    for c in range(N_CORES):
        yt = res.results[c]["yt"]  # [128, NC_RAYS // 2]
        # Per supertile t: cols [t*512:(t+1)*512]; rows 64:128 = A rays
        # (rays t*1024 .. +512), rows 0:64 = B rays (last 512).
        n_super = NC_RAYS // SUPER
        a = yt[OUT:128].reshape(OUT, n_super, B)
        b = yt[0:OUT].reshape(OUT, n_super, B)
        stacked = np.stack([a, b], axis=2)  # [64, t, 2, 512]
        out[c * NC_RAYS : (c + 1) * NC_RAYS] = (
            stacked.transpose(1, 2, 3, 0).reshape(NC_RAYS, OUT)
        )
    return out
